# revision 10
# baseline (speedup 1.0000x reference)
"""GCN message-passing kernel for 8 Trainium2 NeuronCores.

Strategy (dest-sharded pull):
  - Host: add self-loops, compute symmetric degree norms dinv. Nodes are
    assigned to (core, block) bins by a degree-balanced snake-deal so the 8
    cores' per-(block, quartile) edge counts are near-equal. dinv[src] is
    folded into x on the host (input prescale); dinv[dst] is applied on-chip.
  - Launch A: each core computes h' = (dinv*x)_shard @ W_gcn in bf16 from a
    host-pretransposed x^T (no PE transposes); h stored partition-major in
    batches of 7 blocks (14 DMA stores); host untangles for free.
  - Host: assemble full table [100352, 128] bf16 rows (256B gather granule).
  - Launch B: per (superblock, quartile) group, dma_gather the edge chunks'
    source rows over 4 SWDGE queues / 16 DMA engines. Chunks SPAN dest-block
    boundaries inside each group via a core-uniform boundary map (cuts pad
    overhead from 18% to ~3%); boundary chunks get one matmul per touched
    block. Selection matrices are host-built PURE 0/1 one-hots in FP8
    (exact; 1/4 the bytes of bf16 norm-scaled sel). Aggregation is
    TRANSPOSED: aggT[c,d] += matmul(lhsT=G[e,c] bf16, rhs=sel[e,d] fp8).
    Tail per block: DVE multiply by dinv[dst] row, Act relu(+b_gcn bias),
    head yT = W_lin^T @ reluT with a rank-1 b_lin starter matmul, Act copy
    to f32, store out [64, 12544]; host transposes/unpermutes.

The dma_gather int16 index limit (<=32767) forces 4 sub-tables of 25088 rows
(the quartile q). Gather rows are 256B (dma_gather minimum), so per-edge
descriptor count is the hard floor: ~424k descriptors/core at ~22ns/desc
spread over 16 DMA engines (~590us); sel fp8 adds ~190us of DMA time.
"""

import sys
import time as _time

sys.path.insert(0, "/opt/trn_rl_repo")

import numpy as np


def _log(msg):
    print(f"[kernel +{_time.time() - _T0:.1f}s] {msg}", file=sys.stderr, flush=True)


_T0 = _time.time()

N_NODES = 100000
N_EDGES = 3200000
N_FEAT = 256
N_CLASS = 64
N_CORES = 8
NPC = N_NODES // N_CORES          # 12500 dests per core
DBW = 64                          # dest-block width (sel columns)
NB = (NPC + DBW - 1) // DBW       # 196 blocks of 64 dests
NPC_PAD = NB * DBW                # 12544
N_PAD = NPC_PAD * N_CORES         # 100352 table rows
SUB = N_PAD // 4                  # 25088 rows per gather sub-table
P = 128
SBB = 8                           # dest blocks per gather superblock (512 dests)


def _host_prepare(x, edge_index):
    """Sort/pad edges; build index stream + dl/norm blobs + prescaled x^T.

    Returns (S, idx_wrapped, dl_blob, nm_blob, xt_scaled, dinv, tc)."""
    import ml_dtypes
    row = edge_index[0].astype(np.int64)
    col = edge_index[1].astype(np.int64)
    loop = np.arange(N_NODES, dtype=np.int64)
    rows = np.concatenate([row, loop])
    cols = np.concatenate([col, loop])

    deg = np.bincount(col, minlength=N_NODES).astype(np.float32) + 1.0
    dinv = 1.0 / np.sqrt(deg)

    # balanced dest assignment: snake-deal nodes by degree into 1568 groups,
    # then deal groups (sorted by load) round-robin to (core, block) so the
    # 8 cores' per-(b, q) counts are near-equal (shrinks max-over-core pad).
    ngrp = N_CORES * NB
    deg_i = deg.astype(np.int64)
    order_n = np.argsort(-deg_i, kind="stable")
    posn = np.arange(N_NODES) % (2 * ngrp)
    snake = np.where(posn < ngrp, posn, 2 * ngrp - 1 - posn)
    grp_of = np.empty(N_NODES, np.int64)
    grp_of[order_n] = snake
    gtot = np.bincount(grp_of, weights=deg, minlength=ngrp)
    gorder = np.argsort(-gtot, kind="stable")
    grp_core = np.empty(ngrp, np.int64)
    grp_blk = np.empty(ngrp, np.int64)
    grp_core[gorder] = np.arange(ngrp) % N_CORES
    grp_blk[gorder] = np.arange(ngrp) // N_CORES
    # slot within group = rank by node id
    order_g = np.lexsort((np.arange(N_NODES), grp_of))
    rank = np.empty(N_NODES, np.int64)
    gstart = np.concatenate([[0], np.cumsum(np.bincount(grp_of, minlength=ngrp))])
    rank[order_g] = np.arange(N_NODES) - gstart[grp_of[order_g]]
    node_core = grp_core[grp_of]
    node_blk = grp_blk[grp_of]
    node_within = rank

    core = node_core[cols]
    blk = node_blk[cols]
    within = node_within[cols]
    q = rows // SUB
    lidx = (rows % SUB).astype(np.int16)

    sbq = (blk // SBB) * 4 + q          # (superblock, quartile) group id
    nsb = (NB + SBB - 1) // SBB
    key = core * (nsb * 4) + sbq
    order = np.lexsort((blk, key))      # by (core, sb, q) then block
    key_s = key[order]
    lidx_s = lidx[order]
    within_s = within[order]
    blk_s = blk[order]

    # per (core, b, q) counts -> uniform boundary map per (sb, q)
    counts = np.bincount(((core * NB) + blk) * 4 + q,
                         minlength=N_CORES * NB * 4).reshape(N_CORES, NB, 4)
    C = np.zeros((nsb, 4), np.int64)          # chunks per (sb, q) group
    lo = np.zeros((NB, 4), np.int64)          # first chunk of block in group
    hi = np.zeros((NB, 4), np.int64)          # last chunk of block in group
    sstart = np.zeros((N_CORES, NB, 4), np.int64)  # per-core slot start
    for sbi in range(nsb):
        bs = list(range(sbi * SBB, min((sbi + 1) * SBB, NB)))
        for qq in range(4):
            end = np.zeros(N_CORES, np.int64)
            prevB = 0
            for b in bs:
                lo_b = max(prevB - 1, 0)
                st = np.maximum(end, lo_b * P)
                end = st + counts[:, b, qq]
                B_b = int(-(-int(end.max()) // P))
                lo[b, qq] = lo_b
                hi[b, qq] = B_b - 1
                sstart[:, b, qq] = st
                prevB = B_b
            C[sbi, qq] = prevB
    gbase = np.concatenate([[0], np.cumsum((C * P).ravel())])  # per (sb,q)
    tcap = int(gbase[-1])
    tc = tcap // P

    # slot of each edge: group base + per-core block start + rank in block.
    # (core, blk, q) buckets are contiguous under the lexsort; rank = position
    # within the current run.
    bkey_s = ((core[order] * NB) + blk_s) * 4 + q[order]
    n_e = order.size
    first = np.ones(n_e, bool)
    first[1:] = bkey_s[1:] != bkey_s[:-1]
    runstart = np.maximum.accumulate(np.where(first, np.arange(n_e), 0))
    rank = np.arange(n_e, dtype=np.int64) - runstart
    core_s = key_s // (nsb * 4)
    sbq_s = key_s % (nsb * 4)
    slot = gbase[sbq_s] + sstart[core_s, blk_s, q[order]] + rank

    idx_pad = np.zeros((N_CORES, tcap), dtype=np.int16)
    own_pad = np.full((N_CORES, tcap), -1, dtype=np.int64)
    dl_pad = np.zeros((N_CORES, tcap), dtype=np.int64)
    idx_pad[core_s, slot] = lidx_s
    own_pad[core_s, slot] = blk_s
    dl_pad[core_s, slot] = within_s

    # matmul schedule in processing order (b, then q, then chunk lo..hi)
    fp8 = ml_dtypes.float8_e4m3fn
    ent_b, ent_q, ent_ci = [], [], []
    for b in range(NB):
        for qq in range(4):
            for ci in range(int(lo[b, qq]), int(hi[b, qq]) + 1):
                ent_b.append(b); ent_q.append(qq); ent_ci.append(ci)
    n_ent = len(ent_b)
    ent_b = np.array(ent_b); ent_q = np.array(ent_q); ent_ci = np.array(ent_ci)
    # sel blob: one 128xDBW column block per schedule entry
    sel = np.zeros((N_CORES, n_ent, P, DBW), dtype=fp8)
    ent_sbq = (ent_b // SBB) * 4 + ent_q
    ent_slot0 = gbase[ent_sbq] + ent_ci * P   # first slot of entry's chunk
    for k in range(N_CORES):
        for j in range(n_ent):
            s0 = int(ent_slot0[j])
            ownj = own_pad[k, s0:s0 + P]
            m = ownj == ent_b[j]
            if m.any():
                sel[k, j, np.nonzero(m)[0], dl_pad[k, s0:s0 + P][m]] = 1.0
    sel_blob = np.ascontiguousarray(
        sel.transpose(0, 2, 1, 3).reshape(N_CORES, P, n_ent * DBW))
    # per-core dest-side dinv replicated to 64 partitions: [core, 64, NPC_PAD]
    dinv_mat = np.zeros((N_CORES, N_CLASS, NPC_PAD), dtype=np.float32)
    slot_all = node_blk * DBW + node_within
    for k in range(N_CORES):
        m = node_core == k
        dinv_mat[k][:, slot_all[m]] = dinv[m][None, :]

    # gather-call index stream is already in (sb, q, chunk) order
    w = idx_pad.reshape(N_CORES, tcap // 16, 16).transpose(0, 2, 1)
    idx_wrapped = np.tile(w, (1, 8, 1)).copy()

    # prescaled transposed x per core: xT[feat, node] = (x * dinv[:, None]).T
    xt_scaled = np.zeros((N_CORES, N_FEAT, NPC_PAD), dtype=ml_dtypes.bfloat16)
    xs_all = (x * dinv[:, None]).astype(np.float32)
    for k in range(N_CORES):
        sl = xs_all[k * NPC:(k + 1) * NPC]
        xt_scaled[k, :, :sl.shape[0]] = sl.T.astype(ml_dtypes.bfloat16)

    sched = (C, lo, hi, n_ent, tc)
    return (sched, idx_wrapped, sel_blob, dinv_mat, xt_scaled, dinv,
            node_core, node_blk, node_within)


def _build_launch_a():
    import concourse.bacc as bacc
    import concourse.mybir as mybir
    from concourse.tile import TileContext

    nc = bacc.Bacc("TRN2", target_bir_lowering=False, debug=False,
                   num_devices=N_CORES)
    f32 = mybir.dt.float32
    bf16 = mybir.dt.bfloat16
    Copy = mybir.ActivationFunctionType.Copy
    xt_d = nc.dram_tensor("xt", [N_FEAT, NPC_PAD], bf16, kind="ExternalInput")
    w_d = nc.dram_tensor("w", [N_FEAT, N_CLASS], bf16, kind="ExternalInput")
    # h output is partition-major: h[p, b*64+c] = h_row(b*128+p, c); host untangles
    NBA = NPC_PAD // P  # 98 node blocks of 128 (independent of DBW)
    HB = 7  # blocks per store batch (98 = 14*7)
    h_d = nc.dram_tensor("h", [P, NBA * N_CLASS], bf16, kind="ExternalOutput")

    with TileContext(nc) as tc:
        with (
            tc.tile_pool(name="const", bufs=1) as cp,
            tc.tile_pool(name="work", bufs=3) as wp,
            tc.tile_pool(name="ps", bufs=2, space="PSUM") as pp,
        ):
            xt = []
            wt = []
            for k in range(2):
                t = cp.tile([P, NPC_PAD], bf16, tag=f"xt{k}")
                nc.sync.dma_start(out=t[:], in_=xt_d[k * P:(k + 1) * P, :])
                xt.append(t)
                t2 = cp.tile([P, N_CLASS], bf16, tag=f"w{k}")
                nc.sync.dma_start(out=t2[:], in_=w_d[k * P:(k + 1) * P, :])
                wt.append(t2)
            for g in range(NBA // HB):
                hg = wp.tile([P, HB, N_CLASS], bf16, tag="hg")
                for bi in range(HB):
                    i = g * HB + bi
                    ph = pp.tile([P, N_CLASS], f32, tag="ph")
                    for k in range(2):
                        nc.tensor.matmul(ph[:], lhsT=xt[k][:, i * P:(i + 1) * P],
                                         rhs=wt[k][:], start=(k == 0), stop=(k == 1))
                    nc.scalar.activation(hg[:, bi, :], ph[:], Copy)
                nc.sync.dma_start(
                    out=h_d[:, g * HB * N_CLASS:(g + 1) * HB * N_CLASS],
                    in_=hg[:])
    nc.compile()
    return nc


def _build_launch_b(sched):
    C, lo, hi, n_ent, tc_total = sched
    import concourse.bacc as bacc
    import concourse.mybir as mybir
    from concourse.tile import TileContext

    nc = bacc.Bacc("TRN2", target_bir_lowering=False, debug=False,
                   num_devices=N_CORES, num_swdge_queues=4)
    f32 = mybir.dt.float32
    i16 = mybir.dt.int16
    bf16 = mybir.dt.bfloat16
    Relu = mybir.ActivationFunctionType.Relu
    Copy = mybir.ActivationFunctionType.Copy
    mult = mybir.AluOpType.mult

    fp8 = mybir.dt.float8e4
    ncols16 = tc_total * 8  # idx stream columns ([128, tcap/16])
    table_d = nc.dram_tensor("table", [N_PAD, 2 * N_CLASS], bf16, kind="ExternalInput")
    idx_d = nc.dram_tensor("idx", [P, ncols16], i16, kind="ExternalInput")
    sel_d = nc.dram_tensor("sel", [P, n_ent * DBW], fp8, kind="ExternalInput")
    dinvm_d = nc.dram_tensor("dinvm", [N_CLASS, NPC_PAD], f32, kind="ExternalInput")
    wlin_d = nc.dram_tensor("wlin", [N_CLASS, N_CLASS], bf16, kind="ExternalInput")
    bgcn_d = nc.dram_tensor("bgcn", [N_CLASS, 1], f32, kind="ExternalInput")
    blin_d = nc.dram_tensor("blin", [1, N_CLASS], f32, kind="ExternalInput")
    ones_d = nc.dram_tensor("ones", [1, P], f32, kind="ExternalInput")
    out_d = nc.dram_tensor("out", [N_CLASS, NPC_PAD], f32, kind="ExternalOutput")

    nsb = (NB + SBB - 1) // SBB
    # per-block matmul entry counts (sel columns are sequential in sched order)
    nmm = (hi - lo + 1).sum(axis=1)  # [NB]

    with TileContext(nc) as tc:
        with (
            tc.tile_pool(name="const", bufs=1) as cp,
            tc.tile_pool(name="gp", bufs=10) as gp,
            tc.tile_pool(name="oh", bufs=2) as op,
            tc.tile_pool(name="wk", bufs=3) as wp,
            tc.tile_pool(name="pa", bufs=3, space="PSUM") as pa,
            tc.tile_pool(name="pb", bufs=2, space="PSUM") as pb,
        ):
            wlin_t = cp.tile([N_CLASS, N_CLASS], bf16)
            nc.sync.dma_start(out=wlin_t[:], in_=wlin_d[:])
            bgcn_t = cp.tile([N_CLASS, 1], f32)
            nc.sync.dma_start(out=bgcn_t[:], in_=bgcn_d[:])
            blin_t = cp.tile([1, N_CLASS], f32)
            nc.sync.dma_start(out=blin_t[:], in_=blin_d[:])
            ones_t = cp.tile([1, P], f32)
            nc.sync.dma_start(out=ones_t[:], in_=ones_d[:])
            idx_t = cp.tile([P, ncols16], i16, tag="idx")
            nc.scalar.dma_start(out=idx_t[:], in_=idx_d[:])

            qrot = 0
            ioff8 = 0
            j = 0      # global sched entry (b-major: b, q, ci)
            Gt = {}
            for sbi in range(nsb):
                # issue this superblock's 4 gather calls
                for q in range(4):
                    gs = int(C[sbi, q])
                    if gs == 0:
                        continue
                    G = gp.tile([P, gs, 2 * N_CLASS], bf16, tag="G")
                    nc.gpsimd.dma_gather(
                        G[:], table_d[SUB * q:SUB * (q + 1), :],
                        idx_t[:, ioff8:ioff8 + gs * 8],
                        gs * P, gs * P, 2 * N_CLASS,
                        single_packet=False, queue_num=qrot % 4,
                    )
                    qrot += 1
                    ioff8 += gs * 8
                    Gt[(sbi, q)] = G
                blo = sbi * SBB
                bhi = min((sbi + 1) * SBB, NB)
                dvt = op.tile([N_CLASS, (bhi - blo) * DBW], f32, tag="dvt")
                nc.scalar.dma_start(
                    out=dvt[:], in_=dinvm_d[:, blo * DBW:bhi * DBW])
                for b in range(blo, bhi):
                    nmm_b = int(nmm[b])
                    sel_t = wp.tile([P, nmm_b * DBW], fp8, tag="sel")
                    nc.sync.dma_start(
                        out=sel_t[:],
                        in_=sel_d[:, j * DBW:(j + nmm_b) * DBW])
                    pblk = pa.tile([N_CLASS, DBW], f32, tag="pblk")
                    done = 0
                    scol = 0
                    for q in range(4):
                        G = Gt[(sbi, q)]
                        for ci in range(int(lo[b, q]), int(hi[b, q]) + 1):
                            done += 1
                            nc.tensor.matmul(
                                pblk[:],
                                lhsT=G[:, ci, :N_CLASS],
                                rhs=sel_t[:, scol * DBW:(scol + 1) * DBW],
                                start=(done == 1), stop=(done == nmm_b))
                            scol += 1
                    j += nmm_b
                    r1 = wp.tile([N_CLASS, DBW], f32, tag="r1")
                    nc.vector.tensor_tensor(
                        out=r1[:], in0=pblk[:],
                        in1=dvt[:, (b - blo) * DBW:(b - blo + 1) * DBW], op=mult)
                    rt = wp.tile([N_CLASS, DBW], bf16, tag="rt")
                    nc.scalar.activation(rt[:], r1[:], Relu, bias=bgcn_t[:])
                    py = pb.tile([N_CLASS, DBW], f32, tag="py")
                    nc.tensor.matmul(py[:], lhsT=blin_t[:], rhs=ones_t[:, :DBW],
                                     start=True, stop=False)
                    nc.tensor.matmul(py[:], lhsT=wlin_t[:], rhs=rt[:],
                                     start=False, stop=True)
                    ot = wp.tile([N_CLASS, DBW], f32, tag="ot")
                    nc.scalar.activation(ot[:], py[:], Copy)
                    nc.sync.dma_start(out=out_d[:, b * DBW:(b + 1) * DBW],
                                      in_=ot[:])
    nc.compile()
    return nc


def _run(x, edge_index, W_gcn, b_gcn, W_lin, b_lin, trace=False):
    from concourse.bass_utils import run_bass_kernel_spmd
    import ml_dtypes

    x = np.asarray(x, dtype=np.float32)
    edge_index = np.asarray(edge_index)
    W_gcn = np.asarray(W_gcn, dtype=np.float32)
    b_gcn = np.asarray(b_gcn, dtype=np.float32)
    W_lin = np.asarray(W_lin, dtype=np.float32)
    b_lin = np.asarray(b_lin, dtype=np.float32)

    _log("host prepare start")
    (sched, idx_wrapped, sel_blob, dinv_mat, xt_scaled, dinv,
     node_core, node_blk, node_within) = _host_prepare(x, edge_index)
    _log(f"host prepare done, tc_total={sched[4]}, n_ent={sched[3]}")

    # ---- launch A: h' = (dinv*x) @ W_gcn, node-sharded, bf16 ----
    nc_a = _build_launch_a()
    _log("launch A compiled")
    w_bf = W_gcn.astype(ml_dtypes.bfloat16)
    in_maps_a = []
    for k in range(N_CORES):
        in_maps_a.append({"xt": xt_scaled[k], "w": w_bf})
    res_a = run_bass_kernel_spmd(nc_a, in_maps_a, list(range(N_CORES)),
                                 trace=trace)
    _log("launch A ran")
    table = np.zeros((N_PAD, 2 * N_CLASS), dtype=ml_dtypes.bfloat16)
    for k in range(N_CORES):
        hk = res_a.results[k]["h"].reshape(P, NPC_PAD // P, N_CLASS).transpose(1, 0, 2)
        table[k * NPC:(k + 1) * NPC, :N_CLASS] = \
            hk.reshape(NPC_PAD, N_CLASS)[:NPC]

    # ---- launch B: gather + on-chip one-hot aggregate + head ----
    nc_b = _build_launch_b(sched)
    _log("launch B compiled")
    wlin_bf = W_lin.astype(ml_dtypes.bfloat16)
    in_maps_b = []
    for k in range(N_CORES):
        in_maps_b.append({
            "table": table, "idx": idx_wrapped[k],
            "sel": sel_blob[k], "dinvm": dinv_mat[k],
            "wlin": wlin_bf,
            "bgcn": b_gcn[:, None].astype(np.float32),
            "blin": b_lin[None, :].astype(np.float32),
            "ones": np.ones((1, P), np.float32),
        })
    res_b = run_bass_kernel_spmd(nc_b, in_maps_b, list(range(N_CORES)),
                                 trace=trace)
    _log("launch B ran")
    y = np.empty((N_NODES, N_CLASS), np.float32)
    slot = node_blk * DBW + node_within
    for k in range(N_CORES):
        m = node_core == k
        y[m] = res_b.results[k]["out"][:, slot[m]].T.astype(np.float32)
    times = (res_a.exec_time_ns, res_b.exec_time_ns)
    return y, times


def kernel(x, edge_index, W_gcn, b_gcn, W_lin, b_lin):
    y, _ = _run(x, edge_index, W_gcn, b_gcn, W_lin, b_lin, trace=False)
    return y


def kernel_traced(x, edge_index, W_gcn, b_gcn, W_lin, b_lin):
    """Returns (y, (launch_a_ns, launch_b_ns)). Used by test.py."""
    return _run(x, edge_index, W_gcn, b_gcn, W_lin, b_lin, trace=True)


# revision 11
# speedup vs baseline: 1.1130x; 1.1130x over previous
"""GCN message-passing kernel for 8 Trainium2 NeuronCores.

Strategy (dest-sharded pull):
  - Host: add self-loops, compute symmetric degree norms dinv. Nodes are
    assigned to (core, block) bins by a degree-balanced snake-deal so the 8
    cores' per-(block, quartile) edge counts are near-equal. dinv[src] is
    folded into x on the host (input prescale); dinv[dst] is applied on-chip.
  - Launch A: each core computes h' = (dinv*x)_shard @ W_gcn in bf16 from a
    host-pretransposed x^T (no PE transposes); h stored partition-major in
    batches of 7 blocks (14 DMA stores); host untangles for free.
  - Host: assemble full table [100352, 128] bf16 rows (256B gather granule).
  - Launch B: per (superblock, quartile) group, dma_gather the edge chunks'
    source rows over 4 SWDGE queues / 16 DMA engines. Chunks SPAN dest-block
    boundaries inside each group via a core-uniform boundary map (cuts pad
    overhead from 18% to ~3%); boundary chunks get one matmul per touched
    block. Selection matrices are host-built PURE 0/1 one-hots in FP8
    (exact; 1/4 the bytes of bf16 norm-scaled sel). Aggregation is
    TRANSPOSED: aggT[c,d] += matmul(lhsT=G[e,c] bf16, rhs=sel[e,d] fp8).
    Tail per block: DVE multiply by dinv[dst] row, Act relu(+b_gcn bias),
    head yT = W_lin^T @ reluT with a rank-1 b_lin starter matmul, Act copy
    to f32, store out [64, 12544]; host transposes/unpermutes.

The dma_gather int16 index limit (<=32767) forces 4 sub-tables of 25088 rows
(the quartile q). Gather rows are 256B (dma_gather minimum), so per-edge
descriptor count is the hard floor: ~424k descriptors/core at ~22ns/desc
spread over 16 DMA engines (~590us); sel fp8 adds ~190us of DMA time.
"""

import sys
import time as _time

sys.path.insert(0, "/opt/trn_rl_repo")

import numpy as np


def _log(msg):
    print(f"[kernel +{_time.time() - _T0:.1f}s] {msg}", file=sys.stderr, flush=True)


_T0 = _time.time()

N_NODES = 100000
N_EDGES = 3200000
N_FEAT = 256
N_CLASS = 64
N_CORES = 8
NPC = N_NODES // N_CORES          # 12500 dests per core
NB = (NPC + 127) // 128           # 98 blocks of 128 dests
NPC_PAD = NB * 128                # 12544
N_PAD = NPC_PAD * N_CORES         # 100352 table rows
SUB = N_PAD // 4                  # 25088 rows per gather sub-table
P = 128
SBB = 4                           # dest blocks per gather superblock


def _host_prepare(x, edge_index):
    """Sort/pad edges; build index stream + dl/norm blobs + prescaled x^T.

    Returns (S, idx_wrapped, dl_blob, nm_blob, xt_scaled, dinv, tc)."""
    import ml_dtypes
    row = edge_index[0].astype(np.int64)
    col = edge_index[1].astype(np.int64)
    loop = np.arange(N_NODES, dtype=np.int64)
    rows = np.concatenate([row, loop])
    cols = np.concatenate([col, loop])

    deg = np.bincount(col, minlength=N_NODES).astype(np.float32) + 1.0
    dinv = 1.0 / np.sqrt(deg)

    # balanced dest assignment: snake-deal nodes by degree into 784 groups,
    # then deal groups (sorted by load) round-robin to (core, block) so the
    # 8 cores' per-(b, q) counts are near-equal (shrinks max-over-core pad).
    ngrp = N_CORES * NB
    deg_i = deg.astype(np.int64)
    order_n = np.argsort(-deg_i, kind="stable")
    posn = np.arange(N_NODES) % (2 * ngrp)
    snake = np.where(posn < ngrp, posn, 2 * ngrp - 1 - posn)
    grp_of = np.empty(N_NODES, np.int64)
    grp_of[order_n] = snake
    gtot = np.bincount(grp_of, weights=deg, minlength=ngrp)
    gorder = np.argsort(-gtot, kind="stable")
    grp_core = np.empty(ngrp, np.int64)
    grp_blk = np.empty(ngrp, np.int64)
    grp_core[gorder] = np.arange(ngrp) % N_CORES
    grp_blk[gorder] = np.arange(ngrp) // N_CORES
    # slot within group = rank by node id
    order_g = np.lexsort((np.arange(N_NODES), grp_of))
    rank = np.empty(N_NODES, np.int64)
    gstart = np.concatenate([[0], np.cumsum(np.bincount(grp_of, minlength=ngrp))])
    rank[order_g] = np.arange(N_NODES) - gstart[grp_of[order_g]]
    node_core = grp_core[grp_of]
    node_blk = grp_blk[grp_of]
    node_within = rank

    core = node_core[cols]
    blk = node_blk[cols]
    within = node_within[cols]
    q = rows // SUB
    lidx = (rows % SUB).astype(np.int16)

    sbq = (blk // SBB) * 4 + q          # (superblock, quartile) group id
    nsb = (NB + SBB - 1) // SBB
    key = core * (nsb * 4) + sbq
    order = np.lexsort((blk, key))      # by (core, sb, q) then block
    key_s = key[order]
    lidx_s = lidx[order]
    within_s = within[order]
    blk_s = blk[order]

    # per (core, b, q) counts -> uniform boundary map per (sb, q)
    counts = np.bincount(((core * NB) + blk) * 4 + q,
                         minlength=N_CORES * NB * 4).reshape(N_CORES, NB, 4)
    C = np.zeros((nsb, 4), np.int64)          # chunks per (sb, q) group
    lo = np.zeros((NB, 4), np.int64)          # first chunk of block in group
    hi = np.zeros((NB, 4), np.int64)          # last chunk of block in group
    sstart = np.zeros((N_CORES, NB, 4), np.int64)  # per-core slot start
    for sbi in range(nsb):
        bs = list(range(sbi * SBB, min((sbi + 1) * SBB, NB)))
        for qq in range(4):
            end = np.zeros(N_CORES, np.int64)
            prevB = 0
            for b in bs:
                lo_b = max(prevB - 1, 0)
                st = np.maximum(end, lo_b * P)
                end = st + counts[:, b, qq]
                B_b = int(-(-int(end.max()) // P))
                lo[b, qq] = lo_b
                hi[b, qq] = B_b - 1
                sstart[:, b, qq] = st
                prevB = B_b
            C[sbi, qq] = prevB
    gbase = np.concatenate([[0], np.cumsum((C * P).ravel())])  # per (sb,q)
    tcap = int(gbase[-1])
    tc = tcap // P

    # slot of each edge: group base + per-core block start + rank in block.
    # (core, blk, q) buckets are contiguous under the lexsort; rank = position
    # within the current run.
    bkey_s = ((core[order] * NB) + blk_s) * 4 + q[order]
    n_e = order.size
    first = np.ones(n_e, bool)
    first[1:] = bkey_s[1:] != bkey_s[:-1]
    runstart = np.maximum.accumulate(np.where(first, np.arange(n_e), 0))
    rank = np.arange(n_e, dtype=np.int64) - runstart
    core_s = key_s // (nsb * 4)
    sbq_s = key_s % (nsb * 4)
    slot = gbase[sbq_s] + sstart[core_s, blk_s, q[order]] + rank

    idx_pad = np.zeros((N_CORES, tcap), dtype=np.int16)
    own_pad = np.full((N_CORES, tcap), -1, dtype=np.int64)
    dl_pad = np.zeros((N_CORES, tcap), dtype=np.int64)
    idx_pad[core_s, slot] = lidx_s
    own_pad[core_s, slot] = blk_s
    dl_pad[core_s, slot] = within_s

    # matmul schedule in processing order (b, then q, then chunk lo..hi)
    fp8 = ml_dtypes.float8_e4m3fn
    ent_b, ent_q, ent_ci = [], [], []
    for b in range(NB):
        for qq in range(4):
            for ci in range(int(lo[b, qq]), int(hi[b, qq]) + 1):
                ent_b.append(b); ent_q.append(qq); ent_ci.append(ci)
    n_ent = len(ent_b)
    ent_b = np.array(ent_b); ent_q = np.array(ent_q); ent_ci = np.array(ent_ci)
    # sel blob: one 128x128 column block per schedule entry
    sel = np.zeros((N_CORES, n_ent, P, P), dtype=fp8)
    ent_sbq = (ent_b // SBB) * 4 + ent_q
    ent_slot0 = gbase[ent_sbq] + ent_ci * P   # first slot of entry's chunk
    for k in range(N_CORES):
        for j in range(n_ent):
            s0 = int(ent_slot0[j])
            ownj = own_pad[k, s0:s0 + P]
            m = ownj == ent_b[j]
            if m.any():
                sel[k, j, np.nonzero(m)[0], dl_pad[k, s0:s0 + P][m]] = 1.0
    sel_blob = np.ascontiguousarray(
        sel.transpose(0, 2, 1, 3).reshape(N_CORES, P, n_ent * P))
    # per-core dest-side dinv replicated to 64 partitions: [core, 64, NPC_PAD]
    dinv_mat = np.zeros((N_CORES, N_CLASS, NPC_PAD), dtype=np.float32)
    slot_all = node_blk * P + node_within
    for k in range(N_CORES):
        m = node_core == k
        dinv_mat[k][:, slot_all[m]] = dinv[m][None, :]

    # gather-call index stream is already in (sb, q, chunk) order
    w = idx_pad.reshape(N_CORES, tcap // 16, 16).transpose(0, 2, 1)
    idx_wrapped = np.tile(w, (1, 8, 1)).copy()

    # prescaled transposed x per core: xT[feat, node] = (x * dinv[:, None]).T
    xt_scaled = np.zeros((N_CORES, N_FEAT, NPC_PAD), dtype=ml_dtypes.bfloat16)
    xs_all = (x * dinv[:, None]).astype(np.float32)
    for k in range(N_CORES):
        sl = xs_all[k * NPC:(k + 1) * NPC]
        xt_scaled[k, :, :sl.shape[0]] = sl.T.astype(ml_dtypes.bfloat16)

    sched = (C, lo, hi, n_ent, tc)
    return (sched, idx_wrapped, sel_blob, dinv_mat, xt_scaled, dinv,
            node_core, node_blk, node_within)


def _build_launch_a():
    import concourse.bacc as bacc
    import concourse.mybir as mybir
    from concourse.tile import TileContext

    nc = bacc.Bacc("TRN2", target_bir_lowering=False, debug=False,
                   num_devices=N_CORES)
    f32 = mybir.dt.float32
    bf16 = mybir.dt.bfloat16
    Copy = mybir.ActivationFunctionType.Copy
    xt_d = nc.dram_tensor("xt", [N_FEAT, NPC_PAD], bf16, kind="ExternalInput")
    w_d = nc.dram_tensor("w", [N_FEAT, N_CLASS], bf16, kind="ExternalInput")
    # h output is partition-major: h[p, b*64+c] = h_row(b*128+p, c); host untangles
    HB = 7  # blocks per store batch (98 = 14*7)
    h_d = nc.dram_tensor("h", [P, NB * N_CLASS], bf16, kind="ExternalOutput")

    with TileContext(nc) as tc:
        with (
            tc.tile_pool(name="const", bufs=1) as cp,
            tc.tile_pool(name="work", bufs=3) as wp,
            tc.tile_pool(name="ps", bufs=2, space="PSUM") as pp,
        ):
            xt = []
            wt = []
            for k in range(2):
                t = cp.tile([P, NPC_PAD], bf16, tag=f"xt{k}")
                nc.sync.dma_start(out=t[:], in_=xt_d[k * P:(k + 1) * P, :])
                xt.append(t)
                t2 = cp.tile([P, N_CLASS], bf16, tag=f"w{k}")
                nc.sync.dma_start(out=t2[:], in_=w_d[k * P:(k + 1) * P, :])
                wt.append(t2)
            for g in range(NB // HB):
                hg = wp.tile([P, HB, N_CLASS], bf16, tag="hg")
                for bi in range(HB):
                    i = g * HB + bi
                    ph = pp.tile([P, N_CLASS], f32, tag="ph")
                    for k in range(2):
                        nc.tensor.matmul(ph[:], lhsT=xt[k][:, i * P:(i + 1) * P],
                                         rhs=wt[k][:], start=(k == 0), stop=(k == 1))
                    nc.scalar.activation(hg[:, bi, :], ph[:], Copy)
                nc.sync.dma_start(
                    out=h_d[:, g * HB * N_CLASS:(g + 1) * HB * N_CLASS],
                    in_=hg[:])
    nc.compile()
    return nc


def _build_launch_b(sched):
    C, lo, hi, n_ent, tc_total = sched
    import concourse.bacc as bacc
    import concourse.mybir as mybir
    from concourse.tile import TileContext

    nc = bacc.Bacc("TRN2", target_bir_lowering=False, debug=False,
                   num_devices=N_CORES, num_swdge_queues=4)
    f32 = mybir.dt.float32
    i16 = mybir.dt.int16
    bf16 = mybir.dt.bfloat16
    Relu = mybir.ActivationFunctionType.Relu
    Copy = mybir.ActivationFunctionType.Copy
    mult = mybir.AluOpType.mult

    fp8 = mybir.dt.float8e4
    ncols16 = tc_total * 8  # idx stream columns ([128, tcap/16])
    table_d = nc.dram_tensor("table", [N_PAD, 2 * N_CLASS], bf16, kind="ExternalInput")
    idx_d = nc.dram_tensor("idx", [P, ncols16], i16, kind="ExternalInput")
    sel_d = nc.dram_tensor("sel", [P, n_ent * P], fp8, kind="ExternalInput")
    dinvm_d = nc.dram_tensor("dinvm", [N_CLASS, NPC_PAD], f32, kind="ExternalInput")
    wlin_d = nc.dram_tensor("wlin", [N_CLASS, N_CLASS], bf16, kind="ExternalInput")
    bgcn_d = nc.dram_tensor("bgcn", [N_CLASS, 1], f32, kind="ExternalInput")
    blin_d = nc.dram_tensor("blin", [1, N_CLASS], f32, kind="ExternalInput")
    ones_d = nc.dram_tensor("ones", [1, P], f32, kind="ExternalInput")
    out_d = nc.dram_tensor("out", [N_CLASS, NPC_PAD], f32, kind="ExternalOutput")

    nsb = (NB + SBB - 1) // SBB
    # per-block matmul entry counts (sel columns are sequential in sched order)
    nmm = (hi - lo + 1).sum(axis=1)  # [NB]

    with TileContext(nc) as tc:
        with (
            tc.tile_pool(name="const", bufs=1) as cp,
            tc.tile_pool(name="gp", bufs=10) as gp,
            tc.tile_pool(name="oh", bufs=2) as op,
            tc.tile_pool(name="wk", bufs=3) as wp,
            tc.tile_pool(name="pa", bufs=3, space="PSUM") as pa,
            tc.tile_pool(name="pb", bufs=2, space="PSUM") as pb,
        ):
            wlin_t = cp.tile([N_CLASS, N_CLASS], bf16)
            nc.sync.dma_start(out=wlin_t[:], in_=wlin_d[:])
            bgcn_t = cp.tile([N_CLASS, 1], f32)
            nc.sync.dma_start(out=bgcn_t[:], in_=bgcn_d[:])
            blin_t = cp.tile([1, N_CLASS], f32)
            nc.sync.dma_start(out=blin_t[:], in_=blin_d[:])
            ones_t = cp.tile([1, P], f32)
            nc.sync.dma_start(out=ones_t[:], in_=ones_d[:])
            idx_t = cp.tile([P, ncols16], i16, tag="idx")
            nc.scalar.dma_start(out=idx_t[:], in_=idx_d[:])

            qrot = 0
            ioff8 = 0
            j = 0      # global sched entry (b-major: b, q, ci)
            Gt = {}
            for sbi in range(nsb):
                # issue this superblock's 4 gather calls
                for q in range(4):
                    gs = int(C[sbi, q])
                    if gs == 0:
                        continue
                    G = gp.tile([P, gs, 2 * N_CLASS], bf16, tag="G")
                    nc.gpsimd.dma_gather(
                        G[:], table_d[SUB * q:SUB * (q + 1), :],
                        idx_t[:, ioff8:ioff8 + gs * 8],
                        gs * P, gs * P, 2 * N_CLASS,
                        single_packet=False, queue_num=qrot % 4,
                    )
                    qrot += 1
                    ioff8 += gs * 8
                    Gt[(sbi, q)] = G
                blo = sbi * SBB
                bhi = min((sbi + 1) * SBB, NB)
                dvt = op.tile([N_CLASS, (bhi - blo) * P], f32, tag="dvt")
                nc.scalar.dma_start(
                    out=dvt[:], in_=dinvm_d[:, blo * P:bhi * P])
                for b in range(blo, bhi):
                    nmm_b = int(nmm[b])
                    sel_t = wp.tile([P, nmm_b * P], fp8, tag="sel")
                    nc.sync.dma_start(
                        out=sel_t[:],
                        in_=sel_d[:, j * P:(j + nmm_b) * P])
                    pblk = pa.tile([N_CLASS, P], f32, tag="pblk")
                    done = 0
                    scol = 0
                    for q in range(4):
                        G = Gt[(sbi, q)]
                        for ci in range(int(lo[b, q]), int(hi[b, q]) + 1):
                            done += 1
                            nc.tensor.matmul(
                                pblk[:],
                                lhsT=G[:, ci, :N_CLASS],
                                rhs=sel_t[:, scol * P:(scol + 1) * P],
                                start=(done == 1), stop=(done == nmm_b))
                            scol += 1
                    j += nmm_b
                    r1 = wp.tile([N_CLASS, P], f32, tag="r1")
                    nc.vector.tensor_tensor(
                        out=r1[:], in0=pblk[:],
                        in1=dvt[:, (b - blo) * P:(b - blo + 1) * P], op=mult)
                    rt = wp.tile([N_CLASS, P], bf16, tag="rt")
                    nc.scalar.activation(rt[:], r1[:], Relu, bias=bgcn_t[:])
                    py = pb.tile([N_CLASS, P], f32, tag="py")
                    nc.tensor.matmul(py[:], lhsT=blin_t[:], rhs=ones_t[:],
                                     start=True, stop=False)
                    nc.tensor.matmul(py[:], lhsT=wlin_t[:], rhs=rt[:],
                                     start=False, stop=True)
                    ot = wp.tile([N_CLASS, P], f32, tag="ot")
                    nc.scalar.activation(ot[:], py[:], Copy)
                    nc.sync.dma_start(out=out_d[:, b * P:(b + 1) * P], in_=ot[:])
    nc.compile()
    return nc


def _run(x, edge_index, W_gcn, b_gcn, W_lin, b_lin, trace=False):
    from concourse.bass_utils import run_bass_kernel_spmd
    import ml_dtypes

    x = np.asarray(x, dtype=np.float32)
    edge_index = np.asarray(edge_index)
    W_gcn = np.asarray(W_gcn, dtype=np.float32)
    b_gcn = np.asarray(b_gcn, dtype=np.float32)
    W_lin = np.asarray(W_lin, dtype=np.float32)
    b_lin = np.asarray(b_lin, dtype=np.float32)

    _log("host prepare start")
    (sched, idx_wrapped, sel_blob, dinv_mat, xt_scaled, dinv,
     node_core, node_blk, node_within) = _host_prepare(x, edge_index)
    _log(f"host prepare done, tc_total={sched[4]}, n_ent={sched[3]}")

    # ---- launch A: h' = (dinv*x) @ W_gcn, node-sharded, bf16 ----
    nc_a = _build_launch_a()
    _log("launch A compiled")
    w_bf = W_gcn.astype(ml_dtypes.bfloat16)
    in_maps_a = []
    for k in range(N_CORES):
        in_maps_a.append({"xt": xt_scaled[k], "w": w_bf})
    res_a = run_bass_kernel_spmd(nc_a, in_maps_a, list(range(N_CORES)),
                                 trace=trace)
    _log("launch A ran")
    table = np.zeros((N_PAD, 2 * N_CLASS), dtype=ml_dtypes.bfloat16)
    for k in range(N_CORES):
        hk = res_a.results[k]["h"].reshape(P, NB, N_CLASS).transpose(1, 0, 2)
        table[k * NPC:(k + 1) * NPC, :N_CLASS] = \
            hk.reshape(NPC_PAD, N_CLASS)[:NPC]

    # ---- launch B: gather + on-chip one-hot aggregate + head ----
    nc_b = _build_launch_b(sched)
    _log("launch B compiled")
    wlin_bf = W_lin.astype(ml_dtypes.bfloat16)
    in_maps_b = []
    for k in range(N_CORES):
        in_maps_b.append({
            "table": table, "idx": idx_wrapped[k],
            "sel": sel_blob[k], "dinvm": dinv_mat[k],
            "wlin": wlin_bf,
            "bgcn": b_gcn[:, None].astype(np.float32),
            "blin": b_lin[None, :].astype(np.float32),
            "ones": np.ones((1, P), np.float32),
        })
    res_b = run_bass_kernel_spmd(nc_b, in_maps_b, list(range(N_CORES)),
                                 trace=trace)
    _log("launch B ran")
    y = np.empty((N_NODES, N_CLASS), np.float32)
    slot = node_blk * P + node_within
    for k in range(N_CORES):
        m = node_core == k
        y[m] = res_b.results[k]["out"][:, slot[m]].T.astype(np.float32)
    times = (res_a.exec_time_ns, res_b.exec_time_ns)
    return y, times


def kernel(x, edge_index, W_gcn, b_gcn, W_lin, b_lin):
    y, _ = _run(x, edge_index, W_gcn, b_gcn, W_lin, b_lin, trace=False)
    return y


def kernel_traced(x, edge_index, W_gcn, b_gcn, W_lin, b_lin):
    """Returns (y, (launch_a_ns, launch_b_ns)). Used by test.py."""
    return _run(x, edge_index, W_gcn, b_gcn, W_lin, b_lin, trace=True)
